# revision 18
# baseline (speedup 1.0000x reference)
"""Trainium2 Bass kernel for nn_DarcyLoss (data-parallel over batch on 8 cores).

loss = mean((model_output - target)^2)
     + mean_b( 0.5 * (sigma_t/0.01) * mean_hw(F_b^2) )
where F = dx(K * dx p) + dy(K * dy p) + f   (2nd-order finite differences,
K = x0_hat[:,0], p = x0_hat[:,1], f = Darcy source term).

Per-core plan (4 images each, bf16 data path):
 - Work with the scaled stencil G' = 2*G (integer coefficients, exact in bf16).
 - y-derivatives (partition axis): PE matmuls against constant G'^T blocks.
 - x-derivatives (free axis): DVE shifted-AP subtracts + 2-op edge fixups,
   processed two images per op to amortize fixed costs.
 - A (x-part) added into the F' PSUM accumulation via identity matmul.
 - F^2 = Square(0.25*F' + f) fused on ScalarE with accum_out row-sums
   (3 calls per image over one 4-bank PSUM tile).
 - MSE: DVE subtract + ScalarE Square with accum_out, two images per op.
 - Partition reduction: single ones-matmul -> [1,14] partials -> host f64.
"""

import sys
from contextlib import ExitStack

import ml_dtypes
import numpy as np

sys.path.insert(0, "/opt/trn_rl_repo")

import concourse.bass as bass  # noqa: E402
import concourse.tile as tile  # noqa: E402
from concourse import bacc, mybir  # noqa: E402
from concourse import bass_utils  # noqa: E402

N_CORES = 8
B, H, W = 32, 512, 512
BPC = B // N_CORES  # images per core
NPAIR = BPC // 2  # image pairs per core
F32 = mybir.dt.float32
BF16 = mybir.dt.bfloat16

_SUB = mybir.AluOpType.subtract
_ADD = mybir.AluOpType.add
_MUL = mybir.AluOpType.mult
_SQ = mybir.ActivationFunctionType.Square

# accumulator column layout: 4 pde cols per image, then 2 mse cols per pair
ACC_PDE = 4
ACC_MSE0 = BPC * ACC_PDE  # 16
ACC_N = ACC_MSE0 + 2 * NPAIR  # 20


def grad_matrix_2x(n: int) -> np.ndarray:
    """G' = 2 * (torch.gradient, spacing=1, edge_order=2) as a dense matrix."""
    G = np.zeros((n, n), np.float32)
    for h in range(1, n - 1):
        G[h, h + 1] = 1.0
        G[h, h - 1] = -1.0
    G[0, 0], G[0, 1], G[0, 2] = -3.0, 4.0, -1.0
    G[n - 1, n - 1], G[n - 1, n - 2], G[n - 1, n - 3] = 3.0, -4.0, 1.0
    return G


def _stencil_free_axis(nc, dst, src):
    """Apply G' along the last axis: dst/src are [128, nblk, 512] APs."""
    # interior: dst[..., j] = src[..., j+1] - src[..., j-1]
    nc.vector.tensor_tensor(dst[:, :, 1:511], src[:, :, 2:512], src[:, :, 0:510], _SUB)
    # left edge: -3*s0 + 4*s1 - s2  (two fused scalar_tensor_tensor ops)
    nc.vector.scalar_tensor_tensor(
        dst[:, :, 0:1], src[:, :, 0:1], 3.0, src[:, :, 2:3], _MUL, _ADD
    )
    nc.vector.scalar_tensor_tensor(
        dst[:, :, 0:1], src[:, :, 1:2], 4.0, dst[:, :, 0:1], _MUL, _SUB
    )
    # right edge: 3*s511 - 4*s510 + s509
    nc.vector.scalar_tensor_tensor(
        dst[:, :, 511:512], src[:, :, 510:511], 4.0, src[:, :, 509:510], _MUL, _SUB
    )
    nc.vector.scalar_tensor_tensor(
        dst[:, :, 511:512], src[:, :, 511:512], 3.0, dst[:, :, 511:512], _MUL, _SUB
    )


def _kernel_body(ctx, tc, xk, xp, mo, tg, gt, ident, fbias, out):
    nc = tc.nc

    consts = ctx.enter_context(tc.tile_pool(name="consts", bufs=1))
    gt_sb = consts.tile([128, 2048], BF16)
    nc.sync.dma_start(
        gt_sb[:].rearrange("p (jb c) -> p jb c", jb=4),
        gt.rearrange("(jb p) c -> p jb c", p=128),
    )
    id_sb = consts.tile([128, 128], BF16)
    nc.sync.dma_start(id_sb[:], ident)
    fb_sb = consts.tile([128, 2], F32)
    nc.sync.dma_start(fb_sb[:], fbias)
    ones_sb = consts.tile([128, 1], F32)
    nc.vector.memset(ones_sb[:], 1.0)
    acc_sb = consts.tile([128, ACC_N], F32)

    xk_pool = ctx.enter_context(tc.tile_pool(name="xk", bufs=2))
    xp_pool = ctx.enter_context(tc.tile_pool(name="xp", bufs=2))
    t1x_pool = ctx.enter_context(tc.tile_pool(name="t1x", bufs=2))
    t2x_pool = ctx.enter_context(tc.tile_pool(name="t2x", bufs=2))
    a_pool = ctx.enter_context(tc.tile_pool(name="abuf", bufs=2))
    t2y_pool = ctx.enter_context(tc.tile_pool(name="t2y", bufs=2))
    sq_pool = ctx.enter_context(tc.tile_pool(name="sq", bufs=2))
    mo_pool = ctx.enter_context(tc.tile_pool(name="mo", bufs=2))
    tg_pool = ctx.enter_context(tc.tile_pool(name="tg", bufs=2))
    psum = ctx.enter_context(tc.tile_pool(name="psum", bufs=1, space="PSUM"))

    # per-image DMA views for the block-layout pde tensors; the mse tensors
    # are loaded flat (one [128, 8192] tile per image pair, 16KB runs)
    xk_r = xk.rearrange("b (n p) w -> b p n w", p=128)
    xp_r = xp.rearrange("b (n p) w -> b p n w", p=128)
    mo_r = mo.rearrange("(t bi) c (hh hl) w -> t (bi c hh) (hl w)", bi=2, hh=32)
    tg_r = tg.rearrange("(t bi) c (hh hl) w -> t (bi c hh) (hl w)", bi=2, hh=32)

    def gtb(j, i):
        """lhsT block: G'^T[j*128:(j+1)*128, i*128:(i+1)*128]."""
        return gt_sb[:, j * 512 + i * 128 : j * 512 + (i + 1) * 128]

    # prefetch all pde inputs on the gpsimd DMA queue (parallel to sync's)
    kts, pts = [], []
    for pair in range(NPAIR):
        kt = xk_pool.tile([128, 4096], BF16)
        pt = xp_pool.tile([128, 4096], BF16)
        for bi in range(2):
            img = pair * 2 + bi
            ko = bi * 2048
            nc.gpsimd.dma_start(
                kt[:, ko : ko + 2048].rearrange("p (n w) -> p n w", n=4),
                xk_r[img],
            )
            nc.gpsimd.dma_start(
                pt[:, ko : ko + 2048].rearrange("p (n w) -> p n w", n=4),
                xp_r[img],
            )
        kts.append(kt)
        pts.append(pt)

    # ---- MSE for all pairs first: fills the pipeline ramp-up ----
    for pair in range(NPAIR):
        mot = mo_pool.tile([128, 8192], BF16)
        nc.sync.dma_start(mot[:], mo_r[pair])
        tgt_ = tg_pool.tile([128, 8192], BF16)
        nc.sync.dma_start(tgt_[:], tg_r[pair])
        nc.vector.tensor_tensor(mot[:], mot[:], tgt_[:], _SUB)
        for hh in range(2):
            col = ACC_MSE0 + pair * 2 + hh
            nc.scalar.activation(
                mot[:, hh * 4096 : (hh + 1) * 4096],
                mot[:, hh * 4096 : (hh + 1) * 4096],
                _SQ,
                accum_out=acc_sb[:, col : col + 1],
            )

    for pair in range(NPAIR):
        kt = kts[pair]
        pt = pts[pair]

        # ---- x-branch (free axis) on DVE, both images at once ----
        t1x = t1x_pool.tile([128, 4096], BF16)
        _stencil_free_axis(
            nc,
            t1x[:].rearrange("p (m w) -> p m w", m=8),
            pt[:].rearrange("p (m w) -> p m w", m=8),
        )
        t2x = t2x_pool.tile([128, 4096], BF16)
        nc.vector.tensor_tensor(t2x[:], kt[:], t1x[:], _MUL)
        a_t = a_pool.tile([128, 4096], BF16)
        _stencil_free_axis(
            nc,
            a_t[:].rearrange("p (m w) -> p m w", m=8),
            t2x[:].rearrange("p (m w) -> p m w", m=8),
        )

        for bi in range(2):
            img = pair * 2 + bi
            ko = bi * 2048  # this image's offset in the pair tiles

            # ---- y-branch (partition axis) on PE ----
            t1y = psum.tile([128, 2048], F32, tag="t1y")  # 4 banks
            for i in range(4):
                js = [j for j in (i - 1, i, i + 1) if 0 <= j <= 3]
                for k, j in enumerate(js):
                    nc.tensor.matmul(
                        t1y[:, i * 512 : (i + 1) * 512],
                        gtb(j, i),
                        pt[:, ko + j * 512 : ko + (j + 1) * 512],
                        start=(k == 0),
                        stop=(k == len(js) - 1),
                    )
            t2y = t2y_pool.tile([128, 2048], BF16)
            nc.vector.tensor_tensor(t2y[:], kt[:, ko : ko + 2048], t1y[:], _MUL)

            # ---- F' accumulation: two 2-bank PSUM half-tiles per image ----
            c0 = img * ACC_PDE
            for half in range(2):
                fp = psum.tile([128, 1024], F32, tag="fp")
                for ii in range(2):
                    i = half * 2 + ii
                    js = [j for j in (i - 1, i, i + 1) if 0 <= j <= 3]
                    for k, j in enumerate(js):
                        nc.tensor.matmul(
                            fp[:, ii * 512 : (ii + 1) * 512],
                            gtb(j, i),
                            t2y[:, j * 512 : (j + 1) * 512],
                            start=(k == 0),
                            stop=False,
                        )
                    nc.tensor.matmul(
                        fp[:, ii * 512 : (ii + 1) * 512],
                        id_sb[:],
                        a_t[:, ko + i * 512 : ko + (i + 1) * 512],
                        start=False,
                        stop=True,
                    )
                # F^2 = Square(0.25*F' + f) with fused row-sums (2 calls/half)
                sq = sq_pool.tile([128, 1024], F32)
                cc = c0 + half * 2
                if half == 0:
                    nc.scalar.activation(
                        sq[:, 0:64], fp[:, 0:64], _SQ,
                        bias=fb_sb[:, 0:1], scale=0.25,
                        accum_out=acc_sb[:, cc : cc + 1],
                    )
                    nc.scalar.activation(
                        sq[:, 64:1024], fp[:, 64:1024], _SQ, scale=0.25,
                        accum_out=acc_sb[:, cc + 1 : cc + 2],
                    )
                else:
                    nc.scalar.activation(
                        sq[:, 0:960], fp[:, 0:960], _SQ, scale=0.25,
                        accum_out=acc_sb[:, cc : cc + 1],
                    )
                    nc.scalar.activation(
                        sq[:, 960:1024], fp[:, 960:1024], _SQ,
                        bias=fb_sb[:, 1:2], scale=0.25,
                        accum_out=acc_sb[:, cc + 1 : cc + 2],
                    )

    # ---- final partition reduction: [128, ACC_N] -> [1, ACC_N] ----
    red = psum.tile([1, ACC_N], F32, tag="t1y")
    nc.tensor.matmul(red[:], ones_sb[:], acc_sb[:])
    out_sb = consts.tile([1, ACC_N], F32)
    nc.scalar.copy(out_sb[:], red[:])
    nc.sync.dma_start(out, out_sb[:])


_NC_CACHE = {}


def build_program():
    if "nc" in _NC_CACHE:
        return _NC_CACHE["nc"]
    nc = bacc.Bacc(
        "TRN2", target_bir_lowering=False, debug=False, num_devices=N_CORES
    )
    xk = nc.dram_tensor("xk", [BPC, H, W], BF16, kind="ExternalInput").ap()
    xp = nc.dram_tensor("xp", [BPC, H, W], BF16, kind="ExternalInput").ap()
    mo = nc.dram_tensor("mo", [BPC, 2, H, W], BF16, kind="ExternalInput").ap()
    tg = nc.dram_tensor("tg", [BPC, 2, H, W], BF16, kind="ExternalInput").ap()
    gt = nc.dram_tensor("gt", [H, H], BF16, kind="ExternalInput").ap()
    ident = nc.dram_tensor("ident", [128, 128], BF16, kind="ExternalInput").ap()
    fbias = nc.dram_tensor("fbias", [128, 2], F32, kind="ExternalInput").ap()
    out = nc.dram_tensor("partials", [1, ACC_N], F32, kind="ExternalOutput").ap()
    with tile.TileContext(nc) as tc, ExitStack() as ctx:
        _kernel_body(ctx, tc, xk, xp, mo, tg, gt, ident, fbias, out)
    nc.compile()
    _NC_CACHE["nc"] = nc
    return nc


def make_in_maps(model_output, target, x0_hat):
    gt_np = np.ascontiguousarray(grad_matrix_2x(H).T).astype(ml_dtypes.bfloat16)
    id_np = np.eye(128, dtype=np.float32).astype(ml_dtypes.bfloat16)
    fb_np = np.zeros((128, 2), np.float32)
    fb_np[0:64, 0] = 10.0  # f source, rows 0:64 of block n=0 (cols 0:64)
    fb_np[64:128, 1] = -10.0  # f source, rows 448:512 of block n=3 (cols 448:512)
    bf16 = ml_dtypes.bfloat16
    x0_hat = np.asarray(x0_hat, dtype=np.float32)
    mo_b = np.ascontiguousarray(model_output, dtype=np.float32).astype(bf16)
    tg_b = np.ascontiguousarray(target, dtype=np.float32).astype(bf16)
    xk_b = np.ascontiguousarray(x0_hat[:, 0]).astype(bf16)
    xp_b = np.ascontiguousarray(x0_hat[:, 1]).astype(bf16)
    in_maps = []
    for c in range(N_CORES):
        sl = slice(c * BPC, (c + 1) * BPC)
        in_maps.append(
            {
                "xk": xk_b[sl],
                "xp": xp_b[sl],
                "mo": mo_b[sl],
                "tg": tg_b[sl],
                "gt": gt_np,
                "ident": id_np,
                "fbias": fb_np,
            }
        )
    return in_maps


def combine_partials(partials_per_core, sigma_t):
    """partials: per core [1, ACC_N] f32 -> final scalar loss (host f64)."""
    total_mse = 0.0
    total_pde = 0.0
    for c in range(N_CORES):
        p = np.asarray(partials_per_core[c]).reshape(ACC_N).astype(np.float64)
        total_mse += p[ACC_MSE0:ACC_N].sum()
        for img in range(BPC):
            s = p[img * ACC_PDE : (img + 1) * ACC_PDE].sum()
            total_pde += 50.0 * float(sigma_t[c * BPC + img]) * s / float(H * W)
    loss = total_mse / float(B * 2 * H * W) + total_pde / float(B)
    return np.float32(loss)


def kernel(model_output, target, x0_hat, sigma_t):
    nc = build_program()
    in_maps = make_in_maps(model_output, target, x0_hat)
    res = bass_utils.run_bass_kernel_spmd(nc, in_maps, core_ids=list(range(N_CORES)))
    partials = [res.results[c]["partials"] for c in range(N_CORES)]
    return combine_partials(partials, np.asarray(sigma_t))


# revision 19
# speedup vs baseline: 1.3224x; 1.3224x over previous
"""Trainium2 Bass kernel for nn_DarcyLoss (data-parallel over batch on 8 cores).

loss = mean((model_output - target)^2)
     + mean_b( 0.5 * (sigma_t/0.01) * mean_hw(F_b^2) )
where F = dx(K * dx p) + dy(K * dy p) + f   (2nd-order finite differences,
K = x0_hat[:,0], p = x0_hat[:,1], f = Darcy source term).

Per-core plan (4 images each, bf16 data path):
 - Work with the scaled stencil G' = 2*G (integer coefficients, exact in bf16).
 - y-derivatives (partition axis): PE matmuls against constant G'^T blocks.
 - x-derivatives (free axis): DVE shifted-AP subtracts + 2-op edge fixups,
   processed two images per op to amortize fixed costs.
 - A (x-part) added into the F' PSUM accumulation via identity matmul.
 - F^2 = Square(0.25*F' + f) fused on ScalarE with accum_out row-sums
   (3 calls per image over one 4-bank PSUM tile).
 - MSE: DVE subtract + ScalarE Square with accum_out, two images per op.
 - Partition reduction: single ones-matmul -> [1,14] partials -> host f64.
"""

import sys
from contextlib import ExitStack

import ml_dtypes
import numpy as np

sys.path.insert(0, "/opt/trn_rl_repo")

import concourse.bass as bass  # noqa: E402
import concourse.tile as tile  # noqa: E402
from concourse import bacc, mybir  # noqa: E402
from concourse import bass_utils  # noqa: E402

N_CORES = 8
B, H, W = 32, 512, 512
BPC = B // N_CORES  # images per core
NPAIR = BPC // 2  # image pairs per core
F32 = mybir.dt.float32
BF16 = mybir.dt.bfloat16

_SUB = mybir.AluOpType.subtract
_ADD = mybir.AluOpType.add
_MUL = mybir.AluOpType.mult
_SQ = mybir.ActivationFunctionType.Square

# accumulator column layout: 4 pde cols per image, then 2 mse cols per pair
ACC_PDE = 4
ACC_MSE0 = BPC * ACC_PDE  # 16
ACC_N = ACC_MSE0 + 2 * NPAIR  # 20


def grad_matrix_2x(n: int) -> np.ndarray:
    """G' = 2 * (torch.gradient, spacing=1, edge_order=2) as a dense matrix."""
    G = np.zeros((n, n), np.float32)
    for h in range(1, n - 1):
        G[h, h + 1] = 1.0
        G[h, h - 1] = -1.0
    G[0, 0], G[0, 1], G[0, 2] = -3.0, 4.0, -1.0
    G[n - 1, n - 1], G[n - 1, n - 2], G[n - 1, n - 3] = 3.0, -4.0, 1.0
    return G


def _stencil_free_axis(nc, dst, src):
    """Apply G' along the last axis: dst/src are [128, nblk, 512] APs."""
    # interior: dst[..., j] = src[..., j+1] - src[..., j-1]
    nc.vector.tensor_tensor(dst[:, :, 1:511], src[:, :, 2:512], src[:, :, 0:510], _SUB)
    # left edge: -3*s0 + 4*s1 - s2  (two fused scalar_tensor_tensor ops)
    nc.vector.scalar_tensor_tensor(
        dst[:, :, 0:1], src[:, :, 0:1], 3.0, src[:, :, 2:3], _MUL, _ADD
    )
    nc.vector.scalar_tensor_tensor(
        dst[:, :, 0:1], src[:, :, 1:2], 4.0, dst[:, :, 0:1], _MUL, _SUB
    )
    # right edge: 3*s511 - 4*s510 + s509
    nc.vector.scalar_tensor_tensor(
        dst[:, :, 511:512], src[:, :, 510:511], 4.0, src[:, :, 509:510], _MUL, _SUB
    )
    nc.vector.scalar_tensor_tensor(
        dst[:, :, 511:512], src[:, :, 511:512], 3.0, dst[:, :, 511:512], _MUL, _SUB
    )


def _kernel_body(ctx, tc, xk, xp, mo, tg, gt, ident, fbias, out):
    nc = tc.nc

    consts = ctx.enter_context(tc.tile_pool(name="consts", bufs=1))
    gt_sb = consts.tile([128, 2048], BF16)
    nc.sync.dma_start(
        gt_sb[:].rearrange("p (jb c) -> p jb c", jb=4),
        gt.rearrange("(jb p) c -> p jb c", p=128),
    )
    id_sb = consts.tile([128, 128], BF16)
    nc.sync.dma_start(id_sb[:], ident)
    fb_sb = consts.tile([128, 2], F32)
    nc.sync.dma_start(fb_sb[:], fbias)
    ones_sb = consts.tile([128, 1], F32)
    nc.vector.memset(ones_sb[:], 1.0)
    acc_sb = consts.tile([128, ACC_N], F32)

    xk_pool = ctx.enter_context(tc.tile_pool(name="xk", bufs=2))
    xp_pool = ctx.enter_context(tc.tile_pool(name="xp", bufs=2))
    t1x_pool = ctx.enter_context(tc.tile_pool(name="t1x", bufs=2))
    t2x_pool = ctx.enter_context(tc.tile_pool(name="t2x", bufs=2))
    a_pool = ctx.enter_context(tc.tile_pool(name="abuf", bufs=2))
    t2y_pool = ctx.enter_context(tc.tile_pool(name="t2y", bufs=2))
    sq_pool = ctx.enter_context(tc.tile_pool(name="sq", bufs=2))
    mo_pool = ctx.enter_context(tc.tile_pool(name="mo", bufs=2))
    tg_pool = ctx.enter_context(tc.tile_pool(name="tg", bufs=2))
    psum = ctx.enter_context(tc.tile_pool(name="psum", bufs=1, space="PSUM"))

    # per-image DMA views for the block-layout pde tensors; the mse tensors
    # are loaded flat (one [128, 8192] tile per image pair, 16KB runs)
    xk_r = xk.rearrange("b (n p) w -> b p n w", p=128)
    xp_r = xp.rearrange("b (n p) w -> b p n w", p=128)
    mo_r = mo.rearrange("(t bi) c (hh hl) w -> t (bi c hh) (hl w)", bi=2, hh=32)
    tg_r = tg.rearrange("(t bi) c (hh hl) w -> t (bi c hh) (hl w)", bi=2, hh=32)

    def gtb(j, i):
        """lhsT block: G'^T[j*128:(j+1)*128, i*128:(i+1)*128]."""
        return gt_sb[:, j * 512 + i * 128 : j * 512 + (i + 1) * 128]

    # prefetch all pde inputs (block layout for the PE contraction)
    kts, pts = [], []
    for pair in range(NPAIR):
        kt = xk_pool.tile([128, 4096], BF16)
        pt = xp_pool.tile([128, 4096], BF16)
        for bi in range(2):
            img = pair * 2 + bi
            ko = bi * 2048
            nc.sync.dma_start(
                kt[:, ko : ko + 2048].rearrange("p (n w) -> p n w", n=4),
                xk_r[img],
            )
            nc.sync.dma_start(
                pt[:, ko : ko + 2048].rearrange("p (n w) -> p n w", n=4),
                xp_r[img],
            )
        kts.append(kt)
        pts.append(pt)

    # ---- MSE for all pairs first: fills the pipeline ramp-up ----
    for pair in range(NPAIR):
        mot = mo_pool.tile([128, 8192], BF16)
        nc.sync.dma_start(mot[:], mo_r[pair])
        tgt_ = tg_pool.tile([128, 8192], BF16)
        nc.sync.dma_start(tgt_[:], tg_r[pair])
        nc.vector.tensor_tensor(mot[:], mot[:], tgt_[:], _SUB)
        for hh in range(2):
            col = ACC_MSE0 + pair * 2 + hh
            nc.scalar.activation(
                mot[:, hh * 4096 : (hh + 1) * 4096],
                mot[:, hh * 4096 : (hh + 1) * 4096],
                _SQ,
                accum_out=acc_sb[:, col : col + 1],
            )

    for pair in range(NPAIR):
        kt = kts[pair]
        pt = pts[pair]

        # ---- x-branch (free axis) on DVE, both images at once ----
        t1x = t1x_pool.tile([128, 4096], BF16)
        _stencil_free_axis(
            nc,
            t1x[:].rearrange("p (m w) -> p m w", m=8),
            pt[:].rearrange("p (m w) -> p m w", m=8),
        )
        t2x = t2x_pool.tile([128, 4096], BF16)
        nc.vector.tensor_tensor(t2x[:], kt[:], t1x[:], _MUL)
        a_t = a_pool.tile([128, 4096], BF16)
        _stencil_free_axis(
            nc,
            a_t[:].rearrange("p (m w) -> p m w", m=8),
            t2x[:].rearrange("p (m w) -> p m w", m=8),
        )

        for bi in range(2):
            img = pair * 2 + bi
            ko = bi * 2048  # this image's offset in the pair tiles

            # ---- y-branch (partition axis) on PE ----
            t1y = psum.tile([128, 2048], F32, tag="t1y")  # 4 banks
            for i in range(4):
                js = [j for j in (i - 1, i, i + 1) if 0 <= j <= 3]
                for k, j in enumerate(js):
                    nc.tensor.matmul(
                        t1y[:, i * 512 : (i + 1) * 512],
                        gtb(j, i),
                        pt[:, ko + j * 512 : ko + (j + 1) * 512],
                        start=(k == 0),
                        stop=(k == len(js) - 1),
                    )
            t2y = t2y_pool.tile([128, 2048], BF16)
            nc.vector.tensor_tensor(t2y[:], kt[:, ko : ko + 2048], t1y[:], _MUL)

            # ---- F' accumulation: two 2-bank PSUM half-tiles per image ----
            c0 = img * ACC_PDE
            for half in range(2):
                fp = psum.tile([128, 1024], F32, tag="fp")
                for ii in range(2):
                    i = half * 2 + ii
                    js = [j for j in (i - 1, i, i + 1) if 0 <= j <= 3]
                    for k, j in enumerate(js):
                        nc.tensor.matmul(
                            fp[:, ii * 512 : (ii + 1) * 512],
                            gtb(j, i),
                            t2y[:, j * 512 : (j + 1) * 512],
                            start=(k == 0),
                            stop=False,
                        )
                    nc.tensor.matmul(
                        fp[:, ii * 512 : (ii + 1) * 512],
                        id_sb[:],
                        a_t[:, ko + i * 512 : ko + (i + 1) * 512],
                        start=False,
                        stop=True,
                    )
                # F^2 = Square(0.25*F' + f) with fused row-sums (2 calls/half)
                sq = sq_pool.tile([128, 1024], F32)
                cc = c0 + half * 2
                if half == 0:
                    nc.scalar.activation(
                        sq[:, 0:64], fp[:, 0:64], _SQ,
                        bias=fb_sb[:, 0:1], scale=0.25,
                        accum_out=acc_sb[:, cc : cc + 1],
                    )
                    nc.scalar.activation(
                        sq[:, 64:1024], fp[:, 64:1024], _SQ, scale=0.25,
                        accum_out=acc_sb[:, cc + 1 : cc + 2],
                    )
                else:
                    nc.scalar.activation(
                        sq[:, 0:960], fp[:, 0:960], _SQ, scale=0.25,
                        accum_out=acc_sb[:, cc : cc + 1],
                    )
                    nc.scalar.activation(
                        sq[:, 960:1024], fp[:, 960:1024], _SQ,
                        bias=fb_sb[:, 1:2], scale=0.25,
                        accum_out=acc_sb[:, cc + 1 : cc + 2],
                    )

    # ---- final partition reduction: [128, ACC_N] -> [1, ACC_N] ----
    red = psum.tile([1, ACC_N], F32, tag="t1y")
    nc.tensor.matmul(red[:], ones_sb[:], acc_sb[:])
    out_sb = consts.tile([1, ACC_N], F32)
    nc.scalar.copy(out_sb[:], red[:])
    nc.sync.dma_start(out, out_sb[:])


_NC_CACHE = {}


def build_program():
    if "nc" in _NC_CACHE:
        return _NC_CACHE["nc"]
    nc = bacc.Bacc(
        "TRN2", target_bir_lowering=False, debug=False, num_devices=N_CORES
    )
    xk = nc.dram_tensor("xk", [BPC, H, W], BF16, kind="ExternalInput").ap()
    xp = nc.dram_tensor("xp", [BPC, H, W], BF16, kind="ExternalInput").ap()
    mo = nc.dram_tensor("mo", [BPC, 2, H, W], BF16, kind="ExternalInput").ap()
    tg = nc.dram_tensor("tg", [BPC, 2, H, W], BF16, kind="ExternalInput").ap()
    gt = nc.dram_tensor("gt", [H, H], BF16, kind="ExternalInput").ap()
    ident = nc.dram_tensor("ident", [128, 128], BF16, kind="ExternalInput").ap()
    fbias = nc.dram_tensor("fbias", [128, 2], F32, kind="ExternalInput").ap()
    out = nc.dram_tensor("partials", [1, ACC_N], F32, kind="ExternalOutput").ap()
    with tile.TileContext(nc) as tc, ExitStack() as ctx:
        _kernel_body(ctx, tc, xk, xp, mo, tg, gt, ident, fbias, out)
    nc.compile()
    _NC_CACHE["nc"] = nc
    return nc


def make_in_maps(model_output, target, x0_hat):
    gt_np = np.ascontiguousarray(grad_matrix_2x(H).T).astype(ml_dtypes.bfloat16)
    id_np = np.eye(128, dtype=np.float32).astype(ml_dtypes.bfloat16)
    fb_np = np.zeros((128, 2), np.float32)
    fb_np[0:64, 0] = 10.0  # f source, rows 0:64 of block n=0 (cols 0:64)
    fb_np[64:128, 1] = -10.0  # f source, rows 448:512 of block n=3 (cols 448:512)
    bf16 = ml_dtypes.bfloat16
    x0_hat = np.asarray(x0_hat, dtype=np.float32)
    mo_b = np.ascontiguousarray(model_output, dtype=np.float32).astype(bf16)
    tg_b = np.ascontiguousarray(target, dtype=np.float32).astype(bf16)
    xk_b = np.ascontiguousarray(x0_hat[:, 0]).astype(bf16)
    xp_b = np.ascontiguousarray(x0_hat[:, 1]).astype(bf16)
    in_maps = []
    for c in range(N_CORES):
        sl = slice(c * BPC, (c + 1) * BPC)
        in_maps.append(
            {
                "xk": xk_b[sl],
                "xp": xp_b[sl],
                "mo": mo_b[sl],
                "tg": tg_b[sl],
                "gt": gt_np,
                "ident": id_np,
                "fbias": fb_np,
            }
        )
    return in_maps


def combine_partials(partials_per_core, sigma_t):
    """partials: per core [1, ACC_N] f32 -> final scalar loss (host f64)."""
    total_mse = 0.0
    total_pde = 0.0
    for c in range(N_CORES):
        p = np.asarray(partials_per_core[c]).reshape(ACC_N).astype(np.float64)
        total_mse += p[ACC_MSE0:ACC_N].sum()
        for img in range(BPC):
            s = p[img * ACC_PDE : (img + 1) * ACC_PDE].sum()
            total_pde += 50.0 * float(sigma_t[c * BPC + img]) * s / float(H * W)
    loss = total_mse / float(B * 2 * H * W) + total_pde / float(B)
    return np.float32(loss)


def kernel(model_output, target, x0_hat, sigma_t):
    nc = build_program()
    in_maps = make_in_maps(model_output, target, x0_hat)
    res = bass_utils.run_bass_kernel_spmd(nc, in_maps, core_ids=list(range(N_CORES)))
    partials = [res.results[c]["partials"] for c in range(N_CORES)]
    return combine_partials(partials, np.asarray(sigma_t))


# revision 24
# speedup vs baseline: 1.6567x; 1.2528x over previous
"""Trainium2 Bass kernel for nn_DarcyLoss (data-parallel over batch on 8 cores).

loss = mean((model_output - target)^2)
     + mean_b( 0.5 * (sigma_t/0.01) * mean_hw(F_b^2) )
where F = dx(K * dx p) + dy(K * dy p) + f   (2nd-order finite differences,
K = x0_hat[:,0], p = x0_hat[:,1], f = Darcy source term).

Per-core plan (4 images each):
 - Work with the scaled stencil G' = 2*G (integer coefficients, exact in f32).
 - y-derivatives (partition axis): PE matmuls against constant G'^T blocks.
 - x-derivatives (free axis): DVE shifted-AP subtracts + 2-op edge fixups.
 - A (x-part) added into the F' PSUM accumulation via identity matmul.
 - F^2 = Square(0.25*F' + f) fused on ScalarE with accum_out row-sums.
 - MSE: DVE subtract (in-place), ScalarE Square with accum_out.
 - Partition reduction: single ones-matmul -> [1,32] partials -> host f64.
"""

import sys
from contextlib import ExitStack

import ml_dtypes
import numpy as np

sys.path.insert(0, "/opt/trn_rl_repo")

import concourse.bass as bass  # noqa: E402
import concourse.tile as tile  # noqa: E402
from concourse import bacc, mybir  # noqa: E402
from concourse import bass_utils  # noqa: E402

N_CORES = 8
B, H, W = 32, 512, 512
BPC = B // N_CORES  # images per core
F32 = mybir.dt.float32
BF16 = mybir.dt.bfloat16

_SUB = mybir.AluOpType.subtract
_ADD = mybir.AluOpType.add
_MUL = mybir.AluOpType.mult
_SQ = mybir.ActivationFunctionType.Square


def grad_matrix_2x(n: int) -> np.ndarray:
    """G' = 2 * (torch.gradient, spacing=1, edge_order=2) as a dense matrix."""
    G = np.zeros((n, n), np.float32)
    for h in range(1, n - 1):
        G[h, h + 1] = 1.0
        G[h, h - 1] = -1.0
    G[0, 0], G[0, 1], G[0, 2] = -3.0, 4.0, -1.0
    G[n - 1, n - 1], G[n - 1, n - 2], G[n - 1, n - 3] = 3.0, -4.0, 1.0
    return G


def _stencil_free_axis(nc, dst, src):
    """Apply G' along the last (free) axis: dst/src are [128, 4, 512] APs."""
    dre = dst
    # interior: dst[..., j] = src[..., j+1] - src[..., j-1]
    nc.vector.tensor_tensor(dst[:, :, 1:511], src[:, :, 2:512], src[:, :, 0:510], _SUB)
    # left edge: -3*s0 + 4*s1 - s2  (two fused scalar_tensor_tensor ops)
    nc.vector.scalar_tensor_tensor(
        dst[:, :, 0:1], src[:, :, 0:1], 3.0, src[:, :, 2:3], _MUL, _ADD
    )
    nc.vector.scalar_tensor_tensor(
        dst[:, :, 0:1], src[:, :, 1:2], 4.0, dre[:, :, 0:1], _MUL, _SUB
    )
    # right edge: 3*s511 - 4*s510 + s509
    nc.vector.scalar_tensor_tensor(
        dst[:, :, 511:512], src[:, :, 510:511], 4.0, src[:, :, 509:510], _MUL, _SUB
    )
    nc.vector.scalar_tensor_tensor(
        dst[:, :, 511:512], src[:, :, 511:512], 3.0, dre[:, :, 511:512], _MUL, _SUB
    )


def _kernel_body(ctx, tc, xk, xp, mo, tg, gt, ident, fbias, out):
    nc = tc.nc

    consts = ctx.enter_context(tc.tile_pool(name="consts", bufs=1))
    gt_sb = consts.tile([128, 2048], BF16)
    nc.sync.dma_start(
        gt_sb[:].rearrange("p (jb c) -> p jb c", jb=4),
        gt.rearrange("(jb p) c -> p jb c", p=128),
    )
    id_sb = consts.tile([128, 128], BF16)
    nc.sync.dma_start(id_sb[:], ident)
    fb_sb = consts.tile([128, 2], F32)
    nc.sync.dma_start(fb_sb[:], fbias)
    ones_sb = consts.tile([128, 1], F32)
    nc.vector.memset(ones_sb[:], 1.0)
    acc_sb = consts.tile([128, 32], F32)

    xk_pool = ctx.enter_context(tc.tile_pool(name="xk", bufs=2))
    xp_pool = ctx.enter_context(tc.tile_pool(name="xp", bufs=2))
    t1x_pool = ctx.enter_context(tc.tile_pool(name="t1x", bufs=2))
    t2x_pool = ctx.enter_context(tc.tile_pool(name="t2x", bufs=2))
    a_pool = ctx.enter_context(tc.tile_pool(name="abuf", bufs=2))
    t2y_pool = ctx.enter_context(tc.tile_pool(name="t2y", bufs=2))
    sq_pool = ctx.enter_context(tc.tile_pool(name="sq", bufs=3))
    mo_pool = ctx.enter_context(tc.tile_pool(name="mo", bufs=3))
    tg_pool = ctx.enter_context(tc.tile_pool(name="tg", bufs=3))
    p_t1y = ctx.enter_context(tc.tile_pool(name="pt1y", bufs=1, space="PSUM"))
    p_fp = ctx.enter_context(tc.tile_pool(name="pfp", bufs=2, space="PSUM"))
    p_red = ctx.enter_context(tc.tile_pool(name="pred", bufs=1, space="PSUM"))

    # one [128,4096] mse tile per image (both channels), 8KB runs per partition
    mo_r = mo.rearrange("b c (hh hl) w -> b (c hh) (hl w)", hh=64)
    tg_r = tg.rearrange("b c (hh hl) w -> b (c hh) (hl w)", hh=64)

    def gtb(j, i):
        """lhsT block: G'^T[j*128:(j+1)*128, i*128:(i+1)*128]."""
        return gt_sb[:, j * 512 + i * 128 : j * 512 + (i + 1) * 128]

    for img in range(BPC):
        kt = xk_pool.tile([128, 2048], BF16)
        nc.sync.dma_start(
            kt[:].rearrange("p (n w) -> p n w", n=4),
            xk[img].rearrange("(n p) w -> p n w", p=128),
        )
        pt = xp_pool.tile([128, 2048], BF16)
        nc.sync.dma_start(
            pt[:].rearrange("p (n w) -> p n w", n=4),
            xp[img].rearrange("(n p) w -> p n w", p=128),
        )
        Kf = kt[:]
        Pf = pt[:]
        Pv = Pf.rearrange("p (n w) -> p n w", n=4)

        # ---- x-branch (free axis) on DVE ----
        t1x = t1x_pool.tile([128, 2048], BF16)
        _stencil_free_axis(nc, t1x[:].rearrange("p (n w) -> p n w", n=4), Pv)
        t2x = t2x_pool.tile([128, 2048], BF16)
        nc.vector.tensor_tensor(t2x[:], Kf, t1x[:], _MUL)
        a_t = a_pool.tile([128, 2048], BF16)
        _stencil_free_axis(
            nc,
            a_t[:].rearrange("p (n w) -> p n w", n=4),
            t2x[:].rearrange("p (n w) -> p n w", n=4),
        )

        # ---- y-branch (partition axis) on PE ----
        t1y = p_t1y.tile([128, 2048], F32)  # 4 banks, one 512-col chunk per i
        for i in range(4):
            js = [j for j in (i - 1, i, i + 1) if 0 <= j <= 3]
            for k, j in enumerate(js):
                nc.tensor.matmul(
                    t1y[:, i * 512 : (i + 1) * 512],
                    gtb(j, i),
                    Pf[:, j * 512 : (j + 1) * 512],
                    start=(k == 0),
                    stop=(k == len(js) - 1),
                )
        t2y = t2y_pool.tile([128, 2048], BF16)
        nc.vector.tensor_tensor(t2y[:], Kf, t1y[:], _MUL)

        # ---- F' accumulation + fused square/row-sum ----
        c0 = img * 6
        for i in range(4):
            js = [j for j in (i - 1, i, i + 1) if 0 <= j <= 3]
            fp = p_fp.tile([128, 512], F32)
            for k, j in enumerate(js):
                nc.tensor.matmul(
                    fp[:],
                    gtb(j, i),
                    t2y[:, j * 512 : (j + 1) * 512],
                    start=(k == 0),
                    stop=False,
                )
            nc.tensor.matmul(
                fp[:], id_sb[:], a_t[:, i * 512 : (i + 1) * 512], start=False, stop=True
            )
            sq = sq_pool.tile([128, 512], F32)
            if i == 0:
                nc.scalar.activation(
                    sq[:, 0:64], fp[:, 0:64], _SQ,
                    bias=fb_sb[:, 0:1], scale=0.25,
                    accum_out=acc_sb[:, c0 : c0 + 1],
                )
                nc.scalar.activation(
                    sq[:, 64:512], fp[:, 64:512], _SQ, scale=0.25,
                    accum_out=acc_sb[:, c0 + 1 : c0 + 2],
                )
            elif i == 3:
                nc.scalar.activation(
                    sq[:, 0:448], fp[:, 0:448], _SQ, scale=0.25,
                    accum_out=acc_sb[:, c0 + 4 : c0 + 5],
                )
                nc.scalar.activation(
                    sq[:, 448:512], fp[:, 448:512], _SQ,
                    bias=fb_sb[:, 1:2], scale=0.25,
                    accum_out=acc_sb[:, c0 + 5 : c0 + 6],
                )
            else:
                nc.scalar.activation(
                    sq[:], fp[:], _SQ, scale=0.25,
                    accum_out=acc_sb[:, c0 + 1 + i : c0 + 2 + i],
                )

        # ---- MSE (one tile, one sub, two ACT halves); the last image's
        # block is emitted during iter 0 so its squares fill ScalarE's
        # early idle window instead of extending the kernel tail.
        mse_imgs = [img] if img not in (0, BPC - 1) else ([0, BPC - 1] if img == 0 else [])
        for mi in mse_imgs:
            mot = mo_pool.tile([128, 4096], BF16)
            nc.sync.dma_start(mot[:], mo_r[mi])
            tgt_ = tg_pool.tile([128, 4096], BF16)
            nc.sync.dma_start(tgt_[:], tg_r[mi])
            nc.vector.tensor_tensor(mot[:], mot[:], tgt_[:], _SUB)
            for hh in range(2):
                t = 2 * mi + hh
                nc.scalar.activation(
                    mot[:, hh * 2048 : (hh + 1) * 2048],
                    mot[:, hh * 2048 : (hh + 1) * 2048],
                    _SQ,
                    accum_out=acc_sb[:, 24 + t : 25 + t],
                )

    # ---- final partition reduction: [128,32] -> [1,32] ----
    red = p_red.tile([1, 32], F32)
    nc.tensor.matmul(red[:], ones_sb[:], acc_sb[:])
    out_sb = consts.tile([1, 32], F32)
    nc.scalar.copy(out_sb[:], red[:])
    nc.sync.dma_start(out, out_sb[:])


_NC_CACHE = {}


def build_program():
    if "nc" in _NC_CACHE:
        return _NC_CACHE["nc"]
    nc = bacc.Bacc(
        "TRN2", target_bir_lowering=False, debug=False, num_devices=N_CORES
    )
    xk = nc.dram_tensor("xk", [BPC, H, W], BF16, kind="ExternalInput").ap()
    xp = nc.dram_tensor("xp", [BPC, H, W], BF16, kind="ExternalInput").ap()
    mo = nc.dram_tensor("mo", [BPC, 2, H, W], BF16, kind="ExternalInput").ap()
    tg = nc.dram_tensor("tg", [BPC, 2, H, W], BF16, kind="ExternalInput").ap()
    gt = nc.dram_tensor("gt", [H, H], BF16, kind="ExternalInput").ap()
    ident = nc.dram_tensor("ident", [128, 128], BF16, kind="ExternalInput").ap()
    fbias = nc.dram_tensor("fbias", [128, 2], F32, kind="ExternalInput").ap()
    out = nc.dram_tensor("partials", [1, 32], F32, kind="ExternalOutput").ap()
    with tile.TileContext(nc) as tc, ExitStack() as ctx:
        _kernel_body(ctx, tc, xk, xp, mo, tg, gt, ident, fbias, out)
    nc.compile()
    _NC_CACHE["nc"] = nc
    return nc


def make_in_maps(model_output, target, x0_hat):
    gt_np = np.ascontiguousarray(grad_matrix_2x(H).T).astype(ml_dtypes.bfloat16)
    id_np = np.eye(128, dtype=np.float32).astype(ml_dtypes.bfloat16)
    fb_np = np.zeros((128, 2), np.float32)
    fb_np[0:64, 0] = 10.0  # f source, rows 0:64 of block n=0 (cols 0:64)
    fb_np[64:128, 1] = -10.0  # f source, rows 448:512 of block n=3 (cols 448:512)
    bf16 = ml_dtypes.bfloat16
    x0_hat = np.asarray(x0_hat, dtype=np.float32)
    mo_b = np.ascontiguousarray(model_output, dtype=np.float32).astype(bf16)
    tg_b = np.ascontiguousarray(target, dtype=np.float32).astype(bf16)
    xk_b = np.ascontiguousarray(x0_hat[:, 0]).astype(bf16)
    xp_b = np.ascontiguousarray(x0_hat[:, 1]).astype(bf16)
    in_maps = []
    for c in range(N_CORES):
        sl = slice(c * BPC, (c + 1) * BPC)
        in_maps.append(
            {
                "xk": xk_b[sl],
                "xp": xp_b[sl],
                "mo": mo_b[sl],
                "tg": tg_b[sl],
                "gt": gt_np,
                "ident": id_np,
                "fbias": fb_np,
            }
        )
    return in_maps


def combine_partials(partials_per_core, sigma_t):
    """partials: per core [1,32] f32 -> final scalar loss (host f64 math)."""
    total_mse = 0.0
    total_pde = 0.0
    for c in range(N_CORES):
        p = np.asarray(partials_per_core[c]).reshape(32).astype(np.float64)
        total_mse += p[24:32].sum()
        for img in range(BPC):
            s = p[img * 6 : (img + 1) * 6].sum()
            total_pde += 50.0 * float(sigma_t[c * BPC + img]) * s / float(H * W)
    loss = total_mse / float(B * 2 * H * W) + total_pde / float(B)
    return np.float32(loss)


def kernel(model_output, target, x0_hat, sigma_t):
    nc = build_program()
    in_maps = make_in_maps(model_output, target, x0_hat)
    res = bass_utils.run_bass_kernel_spmd(nc, in_maps, core_ids=list(range(N_CORES)))
    partials = [res.results[c]["partials"] for c in range(N_CORES)]
    return combine_partials(partials, np.asarray(sigma_t))
